# revision 31
# baseline (speedup 1.0000x reference)
"""Multi-head attention (B=2, S=4096, DM=512, H=8) on 8 trn2 NeuronCores.

Sharding: data + head parallel. Core c handles batch b = c//4 and head pair
hp = c%4 (heads 2hp, 2hp+1 = a 128-wide slice of the model dim). Each core
computes its two heads' full attention plus the partial output projection
(its 128 rows of Wo); the host sums the 4 partials per batch and adds bo.

v5:
  * Key compaction: the padding mask is known on the host, and masked keys
    get softmax weight exactly 0 in the reference (logit - 1e9). k/v are
    compacted to the unmasked keys (padded to whole 128-key blocks, zero
    rows beyond): NSKP ~ 29 blocks instead of 32.
  * One [128, 1024] logits tile per j (both heads side by side) from a
    3-deep PSUM pool that also serves the Wo partials and the projection
    scratch (every allocation is the same 2-bank shape, so the Wo/proj
    spikes do not steal logits depth unpredictably). Depth 3 hides the
    ~450ns PE-drain+semaphore latency of the QK->exp handoff that bound
    v2 (depth 2) at ~1130ns/j.
  * Whole-tile exp alternating per j: even j on ScalarE (exact spline Exp),
    odd j on the DVE (one-op Schraudolph int16-bitcast fast-exp). The two
    engines work on consecutive j concurrently; one instruction per tile
    keeps the per-instruction constants (352cyc ScalarE / 120cyc DVE) at
    half the per-head-split cost.
  * QK pair and Wo pair issued adjacently with explicit 64x128 tile
    positions (0,0)/(64,0).
  * ScalarE absorbs the acc->stg copies, the K-proj copy, and the
    per-partition rowsum scale of pso0; the DVE keeps the fast-exp, the
    stt combine, the Q-proj copy and the bf16 casts. V-proj mask scaling
    alternates engines (chunk-0 relief).

v6:
  * Input DMA layouts are partition-major (22-28KB contiguous runs per
    partition for the bulk tensors) and issued as ~15 large dma_starts
    ordered by first need (wk,kT0 -> wq,qT0 -> wv,vT0 -> kT1,vT1 ->
    rests). Cuts descriptor count ~4x and un-gates the ch0 JIT feed.
  * Chunk handoff: the acc->stg fp32 copies are gone. DVE casts acc0
    (h0+rowsum row) and ScalarE casts acc1 to bf16 staging in parallel;
    rowsums ride bf16 (0.2% rms on the denominator, well inside budget).
    ScalarE starts the next chunk's exps ~0.8us earlier.
  * All steady-state DMA dispatch (rowsum transposes, outw shift, out
    stores) moved to the sync HWDGE ring so the scalar sequencer only
    dispatches exp/cast work.
"""
import numpy as np
import ml_dtypes

import concourse.bass as bass
from concourse import bacc
import concourse.mybir as mybir
import concourse.tile as tile
from concourse import bass_utils
from concourse.alu_op_type import AluOpType

FP32 = mybir.dt.float32
BF16 = mybir.dt.bfloat16
I16 = mybir.dt.int16
AF = mybir.ActivationFunctionType

B, S, DM, H = 2, 4096, 512, 8
D = DM // H              # 64
NCORES = 8
CHUNK = 512              # q columns processed per attention chunk
NCH = S // CHUNK         # 8
NT = DM // 128           # 4 dm tiles

_CACHE = {}


def _build(with_bias, nskp):
    """nskp = number of 128-key blocks after compaction (<= 32)."""
    kch = (nskp + 3) // 4         # 512-key chunks for K/V (last may be partial)
    nc = bacc.Bacc("TRN2", target_bir_lowering=False, debug=False)

    # partition-major: [p, ch, t, c] so every DMA has long contiguous
    # per-partition runs (4KB per chunk, up to ~24KB for the rest blocks)
    qT0 = nc.dram_tensor("qT0", [128, NT, CHUNK], BF16, kind="ExternalInput")
    qT1 = nc.dram_tensor("qT1", [128, NT, CHUNK], BF16, kind="ExternalInput")
    qTr = nc.dram_tensor("qTr", [128, NCH - 2, NT, CHUNK], BF16,
                         kind="ExternalInput")
    kT0 = nc.dram_tensor("kT0", [128, NT, CHUNK], BF16, kind="ExternalInput")
    kT1 = nc.dram_tensor("kT1", [128, NT, CHUNK], BF16, kind="ExternalInput")
    kTr = nc.dram_tensor("kTr", [128, kch - 2, NT, CHUNK], BF16,
                         kind="ExternalInput")
    vT0 = nc.dram_tensor("vT0", [128, NT, CHUNK], BF16, kind="ExternalInput")
    vT1 = nc.dram_tensor("vT1", [128, NT, CHUNK], BF16, kind="ExternalInput")
    vTr = nc.dram_tensor("vTr", [128, kch - 2, NT, CHUNK], BF16,
                         kind="ExternalInput")
    m01 = nc.dram_tensor("m01", [128, nskp], FP32, kind="ExternalInput")
    wq = nc.dram_tensor("wq", [128, NT, 128], BF16, kind="ExternalInput")
    wk = nc.dram_tensor("wk", [128, NT, 128], BF16, kind="ExternalInput")
    wv = nc.dram_tensor("wv", [128, NT, 130], BF16, kind="ExternalInput")
    bqk = nc.dram_tensor("bqk", [1, 256], BF16, kind="ExternalInput")  # bq|bk
    bv = nc.dram_tensor("bv", [1, 130], BF16, kind="ExternalInput")
    wo = nc.dram_tensor("wo", [128, DM], BF16, kind="ExternalInput")
    out = nc.dram_tensor("out", [S, DM], BF16, kind="ExternalOutput")

    def kblks(g):  # blocks in K/V group g
        return min(4, nskp - 4 * g)

    # wo_combine slots inside the next chunk's j loop: 4 evenly spaced js
    # (starting at 8 so the previous chunk's rowsum transposes, which can
    # queue behind bulk input DMA on the sync ring, have time to land)
    w0 = min(8, nskp - 5)
    gap = max(1, (nskp - 1 - w0) // 3)
    wo_slot = {w0 + i * gap: i for i in range(4)}
    assert len(wo_slot) == 4 and max(wo_slot) <= nskp - 1

    with tile.TileContext(nc) as tc:
        with tc.tile_pool(name="consts", bufs=1) as consts, \
             tc.tile_pool(name="acts", bufs=1) as acts:
            # ---- first chunk's activations land before anything else ----
            qT_sb = acts.tile([128, NCH, NT, CHUNK], BF16)
            kT_sb = acts.tile([128, kch, NT, CHUNK], BF16)
            vT_sb = acts.tile([128, kch, NT, CHUNK], BF16)
            wq_sb = consts.tile([128, NT, 128], BF16)
            wk_sb = consts.tile([128, NT, 128], BF16)
            wv_sb = consts.tile([128, NT, 130], BF16)
            bqk_sb = consts.tile([1, 256], BF16)
            bv_sb = consts.tile([1, 130], BF16)
            wo_sb = consts.tile([128, DM], BF16)     # rows 0:64 h0, 64:128 h1
            wo1_sb = consts.tile([64, DM], BF16)     # h1 rows at partitions 0:64
            m01_sb = consts.tile([128, nskp], FP32)
            # ---- DMAs in strict consumption order. The scalar ring gets
            # ONLY a small early set: HWDGE ring backpressure stalls the
            # issuing sequencer, and the scalar sequencer must stay free to
            # dispatch ch0's copies/exps. Everything bulky goes on the sync
            # ring (its sequencer has no engine duties), pair-granular so
            # per-partition runs are 8KB and arrival is still JIT.
            nc.sync.dma_start(out=kT_sb[:, 0], in_=kT0[:, :, :])
            nc.scalar.dma_start(out=wk_sb, in_=wk[:, :, :])
            nc.scalar.dma_start(out=wv_sb, in_=wv[:, :, :])
            nc.scalar.dma_start(out=m01_sb, in_=m01[:, :])
            nc.sync.dma_start(out=vT_sb[0:64, 0], in_=vT0[0:64, :, :])
            nc.scalar.dma_start(out=vT_sb[64:128, 0], in_=vT0[64:128, :, :])
            nc.sync.dma_start(out=qT_sb[:, 0], in_=qT0[:, :, :])
            nc.sync.dma_start(out=wq_sb, in_=wq[:, :, :])

            # ---- tiny constants; warm the Exp table set during the DMA phase ----
            ones_sb = consts.tile([1, CHUNK], BF16)
            nc.vector.memset(ones_sb, 1.0)
            ones64 = consts.tile([128, 1], BF16)   # 1.0 on every partition
            nc.vector.memset(ones64, 1.0)
            warm = consts.tile([1, 1], FP32)
            nc.scalar.activation(warm, ones_sb[0:1, 0:1], AF.Exp)

            nc.scalar.dma_start(out=bv_sb, in_=bv[:, :])
            if with_bias:
                nc.sync.dma_start(out=bqk_sb, in_=bqk[:, :])

            # ---- bulk k/v/q stream in JIT consumption order (sync ring) ----
            nc.sync.dma_start(out=qT_sb[:, 1], in_=qT1[:, :, :])
            nc.sync.dma_start(out=kT_sb[:, 1], in_=kT1[:, :, :])
            nc.scalar.dma_start(out=vT_sb[:, 1], in_=vT1[:, :, :])
            nc.sync.dma_start(out=wo_sb, in_=wo[:, :])
            nc.sync.dma_start(out=wo1_sb, in_=wo[64:128, :])
            for c0 in range(0, kch - 2, 2):
                c1 = min(c0 + 2, kch - 2)
                nc.sync.dma_start(out=kT_sb[:, 2 + c0:2 + c1], in_=kTr[:, c0:c1])
                nc.sync.dma_start(out=vT_sb[:, 2 + c0:2 + c1], in_=vTr[:, c0:c1])
            for c0 in range(0, NCH - 2, 2):
                c1 = min(c0 + 2, NCH - 2)
                nc.sync.dma_start(out=qT_sb[:, 2 + c0:2 + c1], in_=qTr[:, c0:c1])

            QhT = acts.tile([128, S], BF16)
            KhT = acts.tile([128, kch * CHUNK], BF16)
            VA = acts.tile([128, nskp, 130], BF16)

            with tc.tile_pool(name="pbig", bufs=3, space="PSUM") as pb, \
                 tc.tile_pool(name="pacc", bufs=1, space="PSUM") as pacc, \
                 tc.tile_pool(name="sexp", bufs=6) as sexp, \
                 tc.tile_pool(name="sout", bufs=3) as sout, \
                 tc.tile_pool(name="sow", bufs=2) as sow, \
                 tc.tile_pool(name="srs", bufs=2) as srs:

                def proj_qk(dst, w_sb, brow, x_sb, ch, cols=CHUNK, on_dve=True):
                    psb = pb.tile([128, 2 * CHUNK], FP32, tag="ps")
                    ps = psb[:, 0:CHUNK]
                    sl = bass.ds(ch * CHUNK, cols)
                    for t in range(NT):
                        nc.tensor.matmul(ps[:, 0:cols], w_sb[:, t, :],
                                         x_sb[:, ch, t, 0:cols],
                                         start=(t == 0),
                                         stop=(t == NT - 1 and not with_bias))
                    if with_bias:
                        nc.tensor.matmul(ps[:, 0:cols], brow, ones_sb[:, 0:cols],
                                         start=False, stop=True)
                    if on_dve:
                        nc.vector.tensor_copy(dst[:, sl], ps[:, 0:cols])
                    else:
                        nc.scalar.copy(dst[:, sl], ps[:, 0:cols])

                def proj_k(g):
                    proj_qk(KhT, wk_sb, bqk_sb[0:1, 128:256], kT_sb, g,
                            kblks(g) * 128, on_dve=False)

                def proj_v(j, psv):
                    # (with bias: the bias matmul also writes the ones columns
                    # 64/129 that produce the attention rowsums; without bias
                    # those columns are filled by vones() instead)
                    for t in range(NT):
                        nc.tensor.matmul(psv[:, 0:130],
                                         vT_sb[:, j // 4, t,
                                               (j % 4) * 128:(j % 4 + 1) * 128],
                                         wv_sb[:, t, :],
                                         start=(t == 0),
                                         stop=(t == NT - 1 and not with_bias))
                    if with_bias:
                        nc.tensor.matmul(psv[:, 0:130], ones_sb[0:1, 0:128],
                                         bv_sb, start=False, stop=True)
                    if j % 2 == 0:
                        nc.vector.tensor_scalar(VA[:, j, :], psv[:, 0:130],
                                                m01_sb[:, j:j + 1], None,
                                                op0=AluOpType.mult)
                    else:
                        nc.scalar.mul(VA[:, j, :], psv[:, 0:130],
                                      m01_sb[:, j:j + 1])

                def vones(g):
                    # overwrite the ones columns (64, 129) of group g's VA
                    # blocks with the key-validity mask
                    if with_bias:
                        return
                    jj0, njj = 4 * g, kblks(g)
                    for col in (64, 129):
                        nc.vector.tensor_copy(VA[:, jj0:jj0 + njj, col],
                                              m01_sb[:, jj0:jj0 + njj])

                # Schraudolph fast-exp in bf16 domain (DVE path, odd j):
                # exp(x) ~= bitcast_bf16(int16(x * 2^7/ln2 + (127*2^7 - C)))
                EXP_A = 184.6650292
                EXP_B = float(127 * (1 << 7)) - 5.5918

                def attn_j(j, qsl, acc0, acc1):
                    ctx = tc.high_priority(offset=2000)
                    ctx.__enter__()
                    ksl = bass.ds(j * 128, 128)
                    pt = pb.tile([128, 2 * CHUNK], FP32, tag="ps")
                    nc.tensor.matmul(pt[:, 0:CHUNK],
                                     KhT[0:64, ksl], QhT[0:64, qsl],
                                     start=True, stop=True,
                                     tile_position=(0, 0))
                    nc.tensor.matmul(pt[:, CHUNK:2 * CHUNK],
                                     KhT[64:128, ksl], QhT[64:128, qsl],
                                     start=True, stop=True,
                                     tile_position=(64, 0))
                    # head0 exp on ScalarE (exact), head1 on DVE (fast-exp),
                    # concurrently on the two halves of the same tile
                    et = sexp.tile([128, 2 * CHUNK], BF16, tag="expT")
                    nc.scalar.activation(et[:, 0:CHUNK], pt[:, 0:CHUNK], AF.Exp)
                    eiv = et.bitcast(I16)
                    nc.vector.tensor_scalar(eiv[:, CHUNK:2 * CHUNK],
                                            pt[:, CHUNK:2 * CHUNK],
                                            EXP_A, EXP_B,
                                            op0=AluOpType.mult,
                                            op1=AluOpType.add)
                    nc.tensor.matmul(acc0, VA[:, j, 0:65], et[:, 0:CHUNK],
                                     start=(j == 0), stop=(j == nskp - 1))
                    nc.tensor.matmul(acc1, VA[:, j, 65:130], et[:, CHUNK:2 * CHUNK],
                                     start=(j == 0), stop=(j == nskp - 1))
                    ctx.__exit__(None, None, None)

                def wo_combine(rti, ow0, ch, qt, outw=None, h1src=None):
                    wctx = tc.high_priority(offset=1000)
                    wctx.__enter__()
                    gq = ch * 4 + qt
                    csl = bass.ds(qt * 128, 128)
                    psb = pb.tile([128, 2 * CHUNK], FP32, tag="ps")
                    pso0 = psb[:, 0:CHUNK]
                    pso1 = psb[:, CHUNK:2 * CHUNK]
                    nc.tensor.matmul(pso0, ow0[0:64, csl], wo_sb[0:64, :],
                                     start=True, stop=True,
                                     tile_position=(0, 0))
                    if h1src is None:
                        nc.tensor.matmul(pso1, outw[64:128, csl],
                                         wo_sb[64:128, :],
                                         start=True, stop=True,
                                         tile_position=(64, 0))
                    else:
                        # tail path: head1 staging still at partitions 0:64
                        nc.tensor.matmul(pso1, h1src[0:64, csl], wo1_sb,
                                         start=True, stop=True)
                    tmp = sout.tile([128, DM], FP32, tag="tmp")
                    nc.scalar.mul(tmp, pso0, rti[:, qt:qt + 1])
                    ot = sout.tile([128, DM], BF16, tag="ot")
                    nc.vector.scalar_tensor_tensor(
                        ot, pso1, rti[:, 4 + qt:5 + qt], tmp,
                        op0=AluOpType.mult, op1=AluOpType.add)
                    nc.sync.dma_start(out=out[gq * 128:(gq + 1) * 128, :], in_=ot)
                    wctx.__exit__(None, None, None)

                pending = None
                for ch in range(NCH):
                    qsl = bass.ds(ch * CHUNK, CHUNK)
                    def vgroup(g):
                        jj0 = 4 * g
                        njj = kblks(g)
                        for base in range(jj0, jj0 + njj, 2):
                            psb = pb.tile([128, 2 * CHUNK], FP32, tag="ps")
                            proj_v(base, psb[:, 0:CHUNK])
                            if base + 1 < jj0 + njj:
                                proj_v(base + 1, psb[:, CHUNK:2 * CHUNK])
                        vones(g)

                    if ch == 0:
                        proj_k(0)
                        vgroup(0)
                        proj_qk(QhT, wq_sb, bqk_sb[0:1, 0:128], qT_sb, 0)
                    if ch + 1 < NCH:
                        proj_qk(QhT, wq_sb, bqk_sb[0:1, 0:128], qT_sb, ch + 1)

                    acc0 = pacc.tile([65, CHUNK], FP32, tag="acc0")
                    acc1 = pacc.tile([65, CHUNK], FP32, tag="acc1")
                    for j in range(nskp):
                        if ch == 0 and j % 4 == 0 and j > 0:
                            # feed the rest of the K/V projections just in time
                            g = j // 4
                            proj_k(g)
                            vgroup(g)
                        if pending is not None and j in wo_slot:
                            wo_combine(pending[0], pending[1], pending[2],
                                       wo_slot[j], outw=pending[3])
                        attn_j(j, qsl, acc0, acc1)

                    # release acc banks via parallel direct bf16 casts: rows
                    # 0:64 = attention out, row 64 = rowsums (bf16 rowsum =
                    # ~0.2% rms on the denominator, inside budget). DVE takes
                    # h0, ScalarE h1, so ScalarE reaches the next chunk's
                    # exps ~0.8us sooner than the old 2x fp32 stg copies.
                    pctx = tc.high_priority(offset=2000)
                    pctx.__enter__()
                    ow0 = sow.tile([65, CHUNK], BF16, tag="ow0")
                    ow1 = sow.tile([65, CHUNK], BF16, tag="ow1")
                    nc.vector.tensor_copy(ow0, acc0)
                    nc.scalar.copy(ow1, acc1)
                    # head1 rows to partitions 64:128 so the Wo pair row-tiles
                    outw = sow.tile([128, CHUNK], BF16, tag="outw")
                    if ch + 1 < NCH:
                        nc.sync.dma_start(out=outw[64:128, :], in_=ow1[0:64, :])
                    # transpose rowsums to partitions: rt[p, h*4+qt] = rs_h[qt*128+p]
                    rti = srs.tile([128, 8], FP32, tag="rti")
                    if ch + 1 < NCH:
                        # DMA transpose (sync ring; scalar sequencer stays clean)
                        rt = srs.tile([128, 8], BF16, tag="rt")
                        for h, ow in ((0, ow0), (1, ow1)):
                            for qt in range(4):
                                nc.sync.dma_start(
                                    out=rt[:, h * 4 + qt:h * 4 + qt + 1],
                                    in_=ow[64:65, qt * 128:(qt + 1) * 128])
                        nc.vector.reciprocal(rti, rt)
                    else:
                        # tail: 8 serial ~600ns DMA dispatches would gate the
                        # final Wo combines; transpose on the idle PE instead
                        # (K=1 matmuls: rtp[:, c] = ow_row_slice^T @ [1])
                        rtp = pb.tile([128, 2 * CHUNK], FP32, tag="ps")
                        for h, ow in ((0, ow0), (1, ow1)):
                            for qt in range(4):
                                c = h * 4 + qt
                                nc.tensor.matmul(
                                    rtp[:, c:c + 1],
                                    ow[64:65, qt * 128:(qt + 1) * 128],
                                    ones64[64:65, 0:1],
                                    start=True, stop=True)
                        nc.vector.reciprocal(rti, rtp[:, 0:8])
                    pctx.__exit__(None, None, None)
                    pending = (rti, ow0, ch, outw, ow1)
                for qt in range(4):
                    wo_combine(pending[0], pending[1], pending[2], qt,
                               h1src=pending[4])
    nc.compile()
    return nc


def _prep_core_inputs(c, q, k, v, keep, nskp, Wq, bq, Wk, bk, Wv, bv, Wo):
    b, hp = divmod(c, 4)
    sl = slice(hp * 128, (hp + 1) * 128)
    bf = ml_dtypes.bfloat16
    scale = 1.0 / np.sqrt(np.float32(D))
    kch = (nskp + 3) // 4
    skp = kch * CHUNK

    def packT(x, nch):
        # [Spad, DM] -> transpose -> [128, nch, NT, CHUNK] partition-major
        xt = x.T.reshape(NT, 128, nch, CHUNK).transpose(1, 2, 0, 3)
        return np.ascontiguousarray(xt).astype(bf)

    def packW(w):
        # [DM, m] -> [128, NT, m] partition-major (wsb[p, t, :] = w[t*128+p])
        return np.ascontiguousarray(
            w.reshape(NT, 128, -1).transpose(1, 0, 2)).astype(bf)

    idx = keep[b]
    nkeep = idx.shape[0]
    kc = np.zeros((skp, DM), np.float32)
    vc = np.zeros((skp, DM), np.float32)
    kc[:nkeep] = k[b][idx]
    vc[:nkeep] = v[b][idx]
    qTb = packT(q[b], NCH)
    kTb = packT(kc, kch)
    vTb = packT(vc, kch)
    valid = np.zeros((nskp * 128,), np.float32)
    valid[:nkeep] = 1.0
    m01c = np.ascontiguousarray(valid.reshape(nskp, 128).T).astype(np.float32)

    wq_c = packW(Wq[:, sl] * scale)
    wk_c = packW(Wk[:, sl])
    wvs = Wv[:, sl]
    wv_c = np.zeros((DM, 130), np.float32)
    wv_c[:, 0:64] = wvs[:, 0:64]
    wv_c[:, 65:129] = wvs[:, 64:128]
    wv_c = packW(wv_c)
    bqk_c = np.concatenate([bq[sl] * scale, bk[sl]]).reshape(1, 256).astype(bf)
    bv_c = np.zeros((1, 130), np.float32)
    bv_c[0, 0:64] = bv[sl][0:64]
    bv_c[0, 64] = 1.0
    bv_c[0, 65:129] = bv[sl][64:128]
    bv_c[0, 129] = 1.0
    bv_c = bv_c.astype(bf)
    wo_c = np.ascontiguousarray(Wo[sl, :]).astype(bf)
    return {"qT0": np.ascontiguousarray(qTb[:, 0]),
            "qT1": np.ascontiguousarray(qTb[:, 1]),
            "qTr": np.ascontiguousarray(qTb[:, 2:]),
            "kT0": np.ascontiguousarray(kTb[:, 0]),
            "kT1": np.ascontiguousarray(kTb[:, 1]),
            "kTr": np.ascontiguousarray(kTb[:, 2:]),
            "vT0": np.ascontiguousarray(vTb[:, 0]),
            "vT1": np.ascontiguousarray(vTb[:, 1]),
            "vTr": np.ascontiguousarray(vTb[:, 2:]),
            "m01": m01c, "wq": wq_c, "wk": wk_c,
            "wv": wv_c, "bqk": bqk_c, "bv": bv_c, "wo": wo_c}


LAST_RESULT = None


def kernel(q, k, v, mask, Wq, bq, Wk, bk, Wv, bv, Wo, bo):
    global LAST_RESULT
    f32 = np.float32
    q, k, v, mask = (np.asarray(x, f32) for x in (q, k, v, mask))
    Wq, bq, Wk, bk, Wv, bv, Wo, bo = (
        np.asarray(x, f32) for x in (Wq, bq, Wk, bk, Wv, bv, Wo, bo))

    # compact keys: masked positions have softmax weight exactly 0
    keep = [np.nonzero(mask[b, 0, 0, :] < 0.5)[0] for b in range(B)]
    nskp = max(9, max((len(ix) + 127) // 128 for ix in keep))

    with_bias = bool(np.any(bq) or np.any(bk) or np.any(bv))
    key = ("nc", with_bias, nskp)
    if key not in _CACHE:
        _CACHE[key] = _build(with_bias, nskp)
    nc = _CACHE[key]

    in_maps = [_prep_core_inputs(c, q, k, v, keep, nskp, Wq, bq, Wk, bk, Wv, bv, Wo)
               for c in range(NCORES)]
    res = bass_utils.run_bass_kernel_spmd(nc, in_maps, core_ids=list(range(NCORES)))
    LAST_RESULT = res
    out = np.zeros((B, S, DM), f32)
    for c in range(NCORES):
        out[c // 4] += np.asarray(res.results[c]["out"], f32)
    out += bo
    return out



# revision 32
# speedup vs baseline: 1.2021x; 1.2021x over previous
"""Multi-head attention (B=2, S=4096, DM=512, H=8) on 8 trn2 NeuronCores.

Sharding: data + head parallel. Core c handles batch b = c//4 and head pair
hp = c%4 (heads 2hp, 2hp+1 = a 128-wide slice of the model dim). Each core
computes its two heads' full attention plus the partial output projection
(its 128 rows of Wo); the host sums the 4 partials per batch and adds bo.

v5:
  * Key compaction: the padding mask is known on the host, and masked keys
    get softmax weight exactly 0 in the reference (logit - 1e9). k/v are
    compacted to the unmasked keys (padded to whole 128-key blocks, zero
    rows beyond): NSKP ~ 29 blocks instead of 32.
  * One [128, 1024] logits tile per j (both heads side by side) from a
    3-deep PSUM pool that also serves the Wo partials and the projection
    scratch (every allocation is the same 2-bank shape, so the Wo/proj
    spikes do not steal logits depth unpredictably). Depth 3 hides the
    ~450ns PE-drain+semaphore latency of the QK->exp handoff that bound
    v2 (depth 2) at ~1130ns/j.
  * Whole-tile exp alternating per j: even j on ScalarE (exact spline Exp),
    odd j on the DVE (one-op Schraudolph int16-bitcast fast-exp). The two
    engines work on consecutive j concurrently; one instruction per tile
    keeps the per-instruction constants (352cyc ScalarE / 120cyc DVE) at
    half the per-head-split cost.
  * QK pair and Wo pair issued adjacently with explicit 64x128 tile
    positions (0,0)/(64,0).
  * ScalarE absorbs the acc->stg copies, the K-proj copy, and the
    per-partition rowsum scale of pso0; the DVE keeps the fast-exp, the
    stt combine, the Q-proj copy and the bf16 casts. V-proj mask scaling
    alternates engines (chunk-0 relief).

v6:
  * Input DMA layouts are partition-major (22-28KB contiguous runs per
    partition for the bulk tensors) and issued as ~15 large dma_starts
    ordered by first need (wk,kT0 -> wq,qT0 -> wv,vT0 -> kT1,vT1 ->
    rests). Cuts descriptor count ~4x and un-gates the ch0 JIT feed.
  * Chunk handoff: the acc->stg fp32 copies are gone. DVE casts acc0
    (h0+rowsum row) and ScalarE casts acc1 to bf16 staging in parallel;
    rowsums ride bf16 (0.2% rms on the denominator, well inside budget).
    ScalarE starts the next chunk's exps ~0.8us earlier.
  * All steady-state DMA dispatch (rowsum transposes, outw shift, out
    stores) moved to the sync HWDGE ring so the scalar sequencer only
    dispatches exp/cast work.

v9-v14 (current):
  * Chunk epilogue (bf16 casts, outw shift, rowsum transpose, reciprocal)
    and the whole wo_combine run inside high_priority(2000) scopes; without
    this the reciprocal gets priority-starved behind the next chunk's exps
    for a full chunk and the Wo normalize lands on the critical path
    (dropping wo_combine to priority 1000 costs +58us).
  * Tail rowsum transpose via 8 K=1 PE matmuls into PSUM instead of 8
    serialized ~600ns DMA dispatches (the last chunk has no slack to hide
    them); saves ~5us of tail.
  * wo_slot starts at j=8 so early-chunk rowsum transposes queued behind
    bulk input DMA still land before the first Wo slot.
  * Bulk input arrives chunk-pair-granular on the sync ring in strict JIT
    consumption order; scalar ring only carries a small early set (HWDGE
    ring backpressure blocks the issuing sequencer, so the scalar ring must
    drain before ch0's exp dispatch begins). Quad-chunk slices and
    ring-balancing both measured worse (bursty arrival / sequencer stall).
  * sexp pool depth 6 so exps can run ahead of the AV consumers across
    Wo/Qproj slots.
"""
import numpy as np
import ml_dtypes

import concourse.bass as bass
from concourse import bacc
import concourse.mybir as mybir
import concourse.tile as tile
from concourse import bass_utils
from concourse.alu_op_type import AluOpType

FP32 = mybir.dt.float32
BF16 = mybir.dt.bfloat16
I16 = mybir.dt.int16
AF = mybir.ActivationFunctionType

B, S, DM, H = 2, 4096, 512, 8
D = DM // H              # 64
NCORES = 8
CHUNK = 512              # q columns processed per attention chunk
NCH = S // CHUNK         # 8
NT = DM // 128           # 4 dm tiles

_CACHE = {}


def _build(with_bias, nskp):
    """nskp = number of 128-key blocks after compaction (<= 32)."""
    kch = (nskp + 3) // 4         # 512-key chunks for K/V (last may be partial)
    nc = bacc.Bacc("TRN2", target_bir_lowering=False, debug=False)

    # partition-major: [p, ch, t, c] so every DMA has long contiguous
    # per-partition runs (4KB per chunk, up to ~24KB for the rest blocks)
    qT0 = nc.dram_tensor("qT0", [128, NT, CHUNK], BF16, kind="ExternalInput")
    qT1 = nc.dram_tensor("qT1", [128, NT, CHUNK], BF16, kind="ExternalInput")
    qTr = nc.dram_tensor("qTr", [128, NCH - 2, NT, CHUNK], BF16,
                         kind="ExternalInput")
    kT0 = nc.dram_tensor("kT0", [128, NT, CHUNK], BF16, kind="ExternalInput")
    kT1 = nc.dram_tensor("kT1", [128, NT, CHUNK], BF16, kind="ExternalInput")
    kTr = nc.dram_tensor("kTr", [128, kch - 2, NT, CHUNK], BF16,
                         kind="ExternalInput")
    vT0 = nc.dram_tensor("vT0", [128, NT, CHUNK], BF16, kind="ExternalInput")
    vT1 = nc.dram_tensor("vT1", [128, NT, CHUNK], BF16, kind="ExternalInput")
    vTr = nc.dram_tensor("vTr", [128, kch - 2, NT, CHUNK], BF16,
                         kind="ExternalInput")
    m01 = nc.dram_tensor("m01", [128, nskp], FP32, kind="ExternalInput")
    wq = nc.dram_tensor("wq", [128, NT, 128], BF16, kind="ExternalInput")
    wk = nc.dram_tensor("wk", [128, NT, 128], BF16, kind="ExternalInput")
    wv = nc.dram_tensor("wv", [128, NT, 130], BF16, kind="ExternalInput")
    bqk = nc.dram_tensor("bqk", [1, 256], BF16, kind="ExternalInput")  # bq|bk
    bv = nc.dram_tensor("bv", [1, 130], BF16, kind="ExternalInput")
    wo = nc.dram_tensor("wo", [128, DM], BF16, kind="ExternalInput")
    out = nc.dram_tensor("out", [S, DM], BF16, kind="ExternalOutput")

    def kblks(g):  # blocks in K/V group g
        return min(4, nskp - 4 * g)

    # wo_combine slots inside the next chunk's j loop: 4 evenly spaced js
    # (starting at 8 so the previous chunk's rowsum transposes, which can
    # queue behind bulk input DMA on the sync ring, have time to land)
    w0 = min(8, nskp - 5)
    gap = max(1, (nskp - 1 - w0) // 3)
    wo_slot = {w0 + i * gap: i for i in range(4)}
    assert len(wo_slot) == 4 and max(wo_slot) <= nskp - 1

    with tile.TileContext(nc) as tc:
        with tc.tile_pool(name="consts", bufs=1) as consts, \
             tc.tile_pool(name="acts", bufs=1) as acts:
            # ---- first chunk's activations land before anything else ----
            qT_sb = acts.tile([128, NCH, NT, CHUNK], BF16)
            kT_sb = acts.tile([128, kch, NT, CHUNK], BF16)
            vT_sb = acts.tile([128, kch, NT, CHUNK], BF16)
            wq_sb = consts.tile([128, NT, 128], BF16)
            wk_sb = consts.tile([128, NT, 128], BF16)
            wv_sb = consts.tile([128, NT, 130], BF16)
            bqk_sb = consts.tile([1, 256], BF16)
            bv_sb = consts.tile([1, 130], BF16)
            wo_sb = consts.tile([128, DM], BF16)     # rows 0:64 h0, 64:128 h1
            wo1_sb = consts.tile([64, DM], BF16)     # h1 rows at partitions 0:64
            m01_sb = consts.tile([128, nskp], FP32)
            # ---- DMAs in strict consumption order. The scalar ring gets
            # ONLY a small early set: HWDGE ring backpressure stalls the
            # issuing sequencer, and the scalar sequencer must stay free to
            # dispatch ch0's copies/exps. Everything bulky goes on the sync
            # ring (its sequencer has no engine duties), pair-granular so
            # per-partition runs are 8KB and arrival is still JIT.
            nc.sync.dma_start(out=kT_sb[:, 0], in_=kT0[:, :, :])
            nc.scalar.dma_start(out=wk_sb, in_=wk[:, :, :])
            nc.scalar.dma_start(out=wv_sb, in_=wv[:, :, :])
            nc.scalar.dma_start(out=m01_sb, in_=m01[:, :])
            nc.sync.dma_start(out=vT_sb[0:64, 0], in_=vT0[0:64, :, :])
            nc.scalar.dma_start(out=vT_sb[64:128, 0], in_=vT0[64:128, :, :])
            nc.sync.dma_start(out=qT_sb[:, 0], in_=qT0[:, :, :])
            nc.sync.dma_start(out=wq_sb, in_=wq[:, :, :])

            # ---- tiny constants; warm the Exp table set during the DMA phase ----
            ones_sb = consts.tile([1, CHUNK], BF16)
            nc.vector.memset(ones_sb, 1.0)
            ones64 = consts.tile([128, 1], BF16)   # 1.0 on every partition
            nc.vector.memset(ones64, 1.0)
            warm = consts.tile([1, 1], FP32)
            nc.scalar.activation(warm, ones_sb[0:1, 0:1], AF.Exp)

            nc.scalar.dma_start(out=bv_sb, in_=bv[:, :])
            if with_bias:
                nc.sync.dma_start(out=bqk_sb, in_=bqk[:, :])

            # ---- bulk k/v/q stream in JIT consumption order (sync ring) ----
            nc.sync.dma_start(out=qT_sb[:, 1], in_=qT1[:, :, :])
            nc.sync.dma_start(out=kT_sb[:, 1], in_=kT1[:, :, :])
            nc.scalar.dma_start(out=vT_sb[:, 1], in_=vT1[:, :, :])
            nc.sync.dma_start(out=wo_sb, in_=wo[:, :])
            nc.sync.dma_start(out=wo1_sb, in_=wo[64:128, :])
            for c0 in range(0, kch - 2, 2):
                c1 = min(c0 + 2, kch - 2)
                nc.sync.dma_start(out=kT_sb[:, 2 + c0:2 + c1], in_=kTr[:, c0:c1])
                nc.sync.dma_start(out=vT_sb[:, 2 + c0:2 + c1], in_=vTr[:, c0:c1])
            for c0 in range(0, NCH - 2, 2):
                c1 = min(c0 + 2, NCH - 2)
                nc.sync.dma_start(out=qT_sb[:, 2 + c0:2 + c1], in_=qTr[:, c0:c1])

            QhT = acts.tile([128, S], BF16)
            KhT = acts.tile([128, kch * CHUNK], BF16)
            VA = acts.tile([128, nskp, 130], BF16)

            with tc.tile_pool(name="pbig", bufs=3, space="PSUM") as pb, \
                 tc.tile_pool(name="pacc", bufs=1, space="PSUM") as pacc, \
                 tc.tile_pool(name="sexp", bufs=6) as sexp, \
                 tc.tile_pool(name="sout", bufs=3) as sout, \
                 tc.tile_pool(name="sow", bufs=2) as sow, \
                 tc.tile_pool(name="srs", bufs=2) as srs:

                def proj_qk(dst, w_sb, brow, x_sb, ch, cols=CHUNK, on_dve=True):
                    psb = pb.tile([128, 2 * CHUNK], FP32, tag="ps")
                    ps = psb[:, 0:CHUNK]
                    sl = bass.ds(ch * CHUNK, cols)
                    for t in range(NT):
                        nc.tensor.matmul(ps[:, 0:cols], w_sb[:, t, :],
                                         x_sb[:, ch, t, 0:cols],
                                         start=(t == 0),
                                         stop=(t == NT - 1 and not with_bias))
                    if with_bias:
                        nc.tensor.matmul(ps[:, 0:cols], brow, ones_sb[:, 0:cols],
                                         start=False, stop=True)
                    if on_dve:
                        nc.vector.tensor_copy(dst[:, sl], ps[:, 0:cols])
                    else:
                        nc.scalar.copy(dst[:, sl], ps[:, 0:cols])

                def proj_k(g):
                    proj_qk(KhT, wk_sb, bqk_sb[0:1, 128:256], kT_sb, g,
                            kblks(g) * 128, on_dve=False)

                def proj_v(j, psv):
                    # (with bias: the bias matmul also writes the ones columns
                    # 64/129 that produce the attention rowsums; without bias
                    # those columns are filled by vones() instead)
                    for t in range(NT):
                        nc.tensor.matmul(psv[:, 0:130],
                                         vT_sb[:, j // 4, t,
                                               (j % 4) * 128:(j % 4 + 1) * 128],
                                         wv_sb[:, t, :],
                                         start=(t == 0),
                                         stop=(t == NT - 1 and not with_bias))
                    if with_bias:
                        nc.tensor.matmul(psv[:, 0:130], ones_sb[0:1, 0:128],
                                         bv_sb, start=False, stop=True)
                    if j % 2 == 0:
                        nc.vector.tensor_scalar(VA[:, j, :], psv[:, 0:130],
                                                m01_sb[:, j:j + 1], None,
                                                op0=AluOpType.mult)
                    else:
                        nc.scalar.mul(VA[:, j, :], psv[:, 0:130],
                                      m01_sb[:, j:j + 1])

                def vones(g):
                    # overwrite the ones columns (64, 129) of group g's VA
                    # blocks with the key-validity mask
                    if with_bias:
                        return
                    jj0, njj = 4 * g, kblks(g)
                    for col in (64, 129):
                        nc.vector.tensor_copy(VA[:, jj0:jj0 + njj, col],
                                              m01_sb[:, jj0:jj0 + njj])

                # Schraudolph fast-exp in bf16 domain (DVE path, odd j):
                # exp(x) ~= bitcast_bf16(int16(x * 2^7/ln2 + (127*2^7 - C)))
                EXP_A = 184.6650292
                EXP_B = float(127 * (1 << 7)) - 5.5918

                def attn_j(j, qsl, acc0, acc1):
                    ctx = tc.high_priority(offset=2000)
                    ctx.__enter__()
                    ksl = bass.ds(j * 128, 128)
                    pt = pb.tile([128, 2 * CHUNK], FP32, tag="ps")
                    nc.tensor.matmul(pt[:, 0:CHUNK],
                                     KhT[0:64, ksl], QhT[0:64, qsl],
                                     start=True, stop=True,
                                     tile_position=(0, 0))
                    nc.tensor.matmul(pt[:, CHUNK:2 * CHUNK],
                                     KhT[64:128, ksl], QhT[64:128, qsl],
                                     start=True, stop=True,
                                     tile_position=(64, 0))
                    # head0 exp on ScalarE (exact), head1 on DVE (fast-exp),
                    # concurrently on the two halves of the same tile
                    et = sexp.tile([128, 2 * CHUNK], BF16, tag="expT")
                    nc.scalar.activation(et[:, 0:CHUNK], pt[:, 0:CHUNK], AF.Exp)
                    eiv = et.bitcast(I16)
                    nc.vector.tensor_scalar(eiv[:, CHUNK:2 * CHUNK],
                                            pt[:, CHUNK:2 * CHUNK],
                                            EXP_A, EXP_B,
                                            op0=AluOpType.mult,
                                            op1=AluOpType.add)
                    nc.tensor.matmul(acc0, VA[:, j, 0:65], et[:, 0:CHUNK],
                                     start=(j == 0), stop=(j == nskp - 1))
                    nc.tensor.matmul(acc1, VA[:, j, 65:130], et[:, CHUNK:2 * CHUNK],
                                     start=(j == 0), stop=(j == nskp - 1))
                    ctx.__exit__(None, None, None)

                def wo_combine(rti, ow0, ch, qt, outw=None, h1src=None):
                    wctx = tc.high_priority(offset=2000)
                    wctx.__enter__()
                    gq = ch * 4 + qt
                    csl = bass.ds(qt * 128, 128)
                    psb = pb.tile([128, 2 * CHUNK], FP32, tag="ps")
                    pso0 = psb[:, 0:CHUNK]
                    pso1 = psb[:, CHUNK:2 * CHUNK]
                    nc.tensor.matmul(pso0, ow0[0:64, csl], wo_sb[0:64, :],
                                     start=True, stop=True,
                                     tile_position=(0, 0))
                    if h1src is None:
                        nc.tensor.matmul(pso1, outw[64:128, csl],
                                         wo_sb[64:128, :],
                                         start=True, stop=True,
                                         tile_position=(64, 0))
                    else:
                        # tail path: head1 staging still at partitions 0:64
                        nc.tensor.matmul(pso1, h1src[0:64, csl], wo1_sb,
                                         start=True, stop=True)
                    tmp = sout.tile([128, DM], FP32, tag="tmp")
                    nc.scalar.mul(tmp, pso0, rti[:, qt:qt + 1])
                    ot = sout.tile([128, DM], BF16, tag="ot")
                    nc.vector.scalar_tensor_tensor(
                        ot, pso1, rti[:, 4 + qt:5 + qt], tmp,
                        op0=AluOpType.mult, op1=AluOpType.add)
                    nc.sync.dma_start(out=out[gq * 128:(gq + 1) * 128, :], in_=ot)
                    wctx.__exit__(None, None, None)

                pending = None
                for ch in range(NCH):
                    qsl = bass.ds(ch * CHUNK, CHUNK)
                    def vgroup(g):
                        jj0 = 4 * g
                        njj = kblks(g)
                        for base in range(jj0, jj0 + njj, 2):
                            psb = pb.tile([128, 2 * CHUNK], FP32, tag="ps")
                            proj_v(base, psb[:, 0:CHUNK])
                            if base + 1 < jj0 + njj:
                                proj_v(base + 1, psb[:, CHUNK:2 * CHUNK])
                        vones(g)

                    if ch == 0:
                        proj_k(0)
                        vgroup(0)
                        proj_qk(QhT, wq_sb, bqk_sb[0:1, 0:128], qT_sb, 0)
                    if ch + 1 < NCH:
                        proj_qk(QhT, wq_sb, bqk_sb[0:1, 0:128], qT_sb, ch + 1)

                    acc0 = pacc.tile([65, CHUNK], FP32, tag="acc0")
                    acc1 = pacc.tile([65, CHUNK], FP32, tag="acc1")
                    for j in range(nskp):
                        if ch == 0 and j % 4 == 0 and j > 0:
                            # feed the rest of the K/V projections just in time
                            g = j // 4
                            proj_k(g)
                            vgroup(g)
                        if pending is not None and j in wo_slot:
                            wo_combine(pending[0], pending[1], pending[2],
                                       wo_slot[j], outw=pending[3])
                        attn_j(j, qsl, acc0, acc1)

                    # release acc banks via parallel direct bf16 casts: rows
                    # 0:64 = attention out, row 64 = rowsums (bf16 rowsum =
                    # ~0.2% rms on the denominator, inside budget). DVE takes
                    # h0, ScalarE h1, so ScalarE reaches the next chunk's
                    # exps ~0.8us sooner than the old 2x fp32 stg copies.
                    pctx = tc.high_priority(offset=2000)
                    pctx.__enter__()
                    ow0 = sow.tile([65, CHUNK], BF16, tag="ow0")
                    ow1 = sow.tile([65, CHUNK], BF16, tag="ow1")
                    nc.vector.tensor_copy(ow0, acc0)
                    nc.scalar.copy(ow1, acc1)
                    # head1 rows to partitions 64:128 so the Wo pair row-tiles
                    outw = sow.tile([128, CHUNK], BF16, tag="outw")
                    if ch + 1 < NCH:
                        nc.sync.dma_start(out=outw[64:128, :], in_=ow1[0:64, :])
                    # transpose rowsums to partitions: rt[p, h*4+qt] = rs_h[qt*128+p]
                    rti = srs.tile([128, 8], FP32, tag="rti")
                    if ch + 1 < NCH:
                        # DMA transpose (sync ring; scalar sequencer stays clean)
                        rt = srs.tile([128, 8], BF16, tag="rt")
                        for h, ow in ((0, ow0), (1, ow1)):
                            for qt in range(4):
                                nc.sync.dma_start(
                                    out=rt[:, h * 4 + qt:h * 4 + qt + 1],
                                    in_=ow[64:65, qt * 128:(qt + 1) * 128])
                        nc.vector.reciprocal(rti, rt)
                    else:
                        # tail: 8 serial ~600ns DMA dispatches would gate the
                        # final Wo combines; transpose on the idle PE instead
                        # (K=1 matmuls: rtp[:, c] = ow_row_slice^T @ [1])
                        rtp = pb.tile([128, 2 * CHUNK], FP32, tag="ps")
                        for h, ow in ((0, ow0), (1, ow1)):
                            for qt in range(4):
                                c = h * 4 + qt
                                nc.tensor.matmul(
                                    rtp[:, c:c + 1],
                                    ow[64:65, qt * 128:(qt + 1) * 128],
                                    ones64[64:65, 0:1],
                                    start=True, stop=True)
                        nc.vector.reciprocal(rti, rtp[:, 0:8])
                    pctx.__exit__(None, None, None)
                    pending = (rti, ow0, ch, outw, ow1)
                for qt in range(4):
                    wo_combine(pending[0], pending[1], pending[2], qt,
                               h1src=pending[4])
    nc.compile()
    return nc


def _prep_core_inputs(c, q, k, v, keep, nskp, Wq, bq, Wk, bk, Wv, bv, Wo):
    b, hp = divmod(c, 4)
    sl = slice(hp * 128, (hp + 1) * 128)
    bf = ml_dtypes.bfloat16
    scale = 1.0 / np.sqrt(np.float32(D))
    kch = (nskp + 3) // 4
    skp = kch * CHUNK

    def packT(x, nch):
        # [Spad, DM] -> transpose -> [128, nch, NT, CHUNK] partition-major
        xt = x.T.reshape(NT, 128, nch, CHUNK).transpose(1, 2, 0, 3)
        return np.ascontiguousarray(xt).astype(bf)

    def packW(w):
        # [DM, m] -> [128, NT, m] partition-major (wsb[p, t, :] = w[t*128+p])
        return np.ascontiguousarray(
            w.reshape(NT, 128, -1).transpose(1, 0, 2)).astype(bf)

    idx = keep[b]
    nkeep = idx.shape[0]
    kc = np.zeros((skp, DM), np.float32)
    vc = np.zeros((skp, DM), np.float32)
    kc[:nkeep] = k[b][idx]
    vc[:nkeep] = v[b][idx]
    qTb = packT(q[b], NCH)
    kTb = packT(kc, kch)
    vTb = packT(vc, kch)
    valid = np.zeros((nskp * 128,), np.float32)
    valid[:nkeep] = 1.0
    m01c = np.ascontiguousarray(valid.reshape(nskp, 128).T).astype(np.float32)

    wq_c = packW(Wq[:, sl] * scale)
    wk_c = packW(Wk[:, sl])
    wvs = Wv[:, sl]
    wv_c = np.zeros((DM, 130), np.float32)
    wv_c[:, 0:64] = wvs[:, 0:64]
    wv_c[:, 65:129] = wvs[:, 64:128]
    wv_c = packW(wv_c)
    bqk_c = np.concatenate([bq[sl] * scale, bk[sl]]).reshape(1, 256).astype(bf)
    bv_c = np.zeros((1, 130), np.float32)
    bv_c[0, 0:64] = bv[sl][0:64]
    bv_c[0, 64] = 1.0
    bv_c[0, 65:129] = bv[sl][64:128]
    bv_c[0, 129] = 1.0
    bv_c = bv_c.astype(bf)
    wo_c = np.ascontiguousarray(Wo[sl, :]).astype(bf)
    return {"qT0": np.ascontiguousarray(qTb[:, 0]),
            "qT1": np.ascontiguousarray(qTb[:, 1]),
            "qTr": np.ascontiguousarray(qTb[:, 2:]),
            "kT0": np.ascontiguousarray(kTb[:, 0]),
            "kT1": np.ascontiguousarray(kTb[:, 1]),
            "kTr": np.ascontiguousarray(kTb[:, 2:]),
            "vT0": np.ascontiguousarray(vTb[:, 0]),
            "vT1": np.ascontiguousarray(vTb[:, 1]),
            "vTr": np.ascontiguousarray(vTb[:, 2:]),
            "m01": m01c, "wq": wq_c, "wk": wk_c,
            "wv": wv_c, "bqk": bqk_c, "bv": bv_c, "wo": wo_c}


LAST_RESULT = None


def kernel(q, k, v, mask, Wq, bq, Wk, bk, Wv, bv, Wo, bo):
    global LAST_RESULT
    f32 = np.float32
    q, k, v, mask = (np.asarray(x, f32) for x in (q, k, v, mask))
    Wq, bq, Wk, bk, Wv, bv, Wo, bo = (
        np.asarray(x, f32) for x in (Wq, bq, Wk, bk, Wv, bv, Wo, bo))

    # compact keys: masked positions have softmax weight exactly 0
    keep = [np.nonzero(mask[b, 0, 0, :] < 0.5)[0] for b in range(B)]
    nskp = max(9, max((len(ix) + 127) // 128 for ix in keep))

    with_bias = bool(np.any(bq) or np.any(bk) or np.any(bv))
    key = ("nc", with_bias, nskp)
    if key not in _CACHE:
        _CACHE[key] = _build(with_bias, nskp)
    nc = _CACHE[key]

    in_maps = [_prep_core_inputs(c, q, k, v, keep, nskp, Wq, bq, Wk, bk, Wv, bv, Wo)
               for c in range(NCORES)]
    res = bass_utils.run_bass_kernel_spmd(nc, in_maps, core_ids=list(range(NCORES)))
    LAST_RESULT = res
    out = np.zeros((B, S, DM), f32)
    for c in range(NCORES):
        out[c // 4] += np.asarray(res.results[c]["out"], f32)
    out += bo
    return out

